# revision 5
# baseline (speedup 1.0000x reference)
# GraphSAGE (3-layer, mean aggregation) on 8 Trainium2 NeuronCores.
#
# Sharding: nodes are split into 8 contiguous ranges (6250 per core); edges are
# partitioned by destination node so each core's scatter-adds stay local.  Each
# layer's input features are replicated to every core via AllGather (the x table
# for layer 0 is simply fed to every core), so the per-edge source gathers are
# local HBM reads.
#
# All tables / weights / PE operands are fp16 (full-rate on the PE, 2x on the
# DVE, and ~8x the mantissa precision of bf16).
#
# Each layer runs in TWO PASSES so the inter-layer AllGathers overlap compute:
#   pass A: for each chunk of 128 destination nodes, gather the edges whose
#     sources live in the LOW half of the feature table and accumulate the
#     transposed partial mean  loPart[c][feat, dst]  on the PE (via per-tile
#     selection matrices S[e, j] = (dst_local[e] == j) / deg built on the DVE),
#     parking the partial in SBUF.
#   pass B: gather the HIGH-half edges, accumulate hiPart, then
#     h = relu(loPart^T w_l + hiPart^T w_l + own^T^T w_r (+ b)) with three
#     accumulating matmuls into one PSUM tile; the Activation engine applies
#     relu / drains PSUM copies.
# The low-half AllGather of layer l's output fires mid-way through pass B
# (after the chunks that produce table rows [0, HALFR) complete), and the
# high-half AllGather's latency is hidden by layer l+1's pass A, which only
# needs the low half.
#
# Gather slots are padded (SPMD-uniform tile counts) with index -1: the SWDGE
# ucode skips trailing negative indices, so padding costs no descriptors/HBM
# bandwidth. Selection-matrix columns for padded slots are 0 (dst_local = -1),
# so whatever data sits in the padded SBUF slots contributes nothing.
import os
import sys

import numpy as np

for _p in ("/opt/trn_rl_repo", "/root/.axon_site/_ro/trn_rl_repo"):
    if _p not in sys.path and os.path.isdir(_p):
        sys.path.append(_p)

from concourse import bacc, mybir, tile  # noqa: E402
from concourse.bass_utils import axon_active, run_bass_kernel_spmd  # noqa: E402
from concourse.masks import make_identity  # noqa: E402

P = 128
FP16 = mybir.dt.float16
F32 = mybir.dt.float32
I16 = mybir.dt.int16
NP_FP16 = np.float16
MAXI = 512  # gather indices per SWDGE instruction (descriptor ring is 1024)


class GSCfg:
    """Static problem configuration (shapes shared by all cores)."""

    def __init__(self, n_nodes, n_cores, d_in, d_hid, d_out, half):
        assert n_nodes % n_cores == 0
        self.N = n_nodes
        self.NCORES = n_cores
        self.NPC = n_nodes // n_cores  # nodes per core
        self.D_IN = d_in
        self.D_HID = d_hid
        self.D_OUT = d_out
        # Gather tables are split at row `half` so int16 indices can address
        # each piece. half % n_cores == 0; per-core producer split = HALFR.
        self.HALF = half
        assert half % n_cores == 0
        self.HALFR = half // n_cores
        assert self.HALFR < self.NPC
        assert half <= 2**15 and (n_nodes - half) < 2**15
        self.NCH = (self.NPC + P - 1) // P  # dst chunks per core


def preprocess(cfg: GSCfg, src: np.ndarray, dst: np.ndarray):
    """Partition + sort edges by destination, pad to an SPMD-uniform tile
    layout, and build the per-core SBUF-layout index/selection arrays."""
    N, NPC, NCH, HALF, HALFR = cfg.N, cfg.NPC, cfg.NCH, cfg.HALF, cfg.HALFR
    NC = cfg.NCORES

    deg = np.bincount(dst, minlength=N)
    invdeg_per_node = (1.0 / np.maximum(deg, 1)).astype(np.float32)

    order = np.argsort(dst, kind="stable")
    s_src = src[order]
    s_dst = dst[order]

    # edge index ranges for (core, chunk)
    chunk_lo = np.empty((NC, NCH), dtype=np.int64)
    chunk_hi = np.empty((NC, NCH), dtype=np.int64)
    for i in range(NC):
        for c in range(NCH):
            d0 = i * NPC + c * P
            d1 = i * NPC + min((c + 1) * P, NPC)
            chunk_lo[i, c] = np.searchsorted(s_dst, d0, side="left")
            chunk_hi[i, c] = np.searchsorted(s_dst, d1, side="left")

    # split each chunk's edges into low-src / high-src groups
    lo_cnt = np.zeros((NC, NCH), dtype=np.int64)
    for i in range(NC):
        for c in range(NCH):
            e0, e1 = chunk_lo[i, c], chunk_hi[i, c]
            lo_cnt[i, c] = int(np.count_nonzero((s_src[e0:e1] % NPC) < HALFR))
    hi_cnt = (chunk_hi - chunk_lo) - lo_cnt

    cdiv = lambda a, b: -(-a // b)
    T_low = [int(cdiv(int(lo_cnt[:, c].max()), P)) for c in range(NCH)]
    T_high = [int(cdiv(int(hi_cnt[:, c].max()), P)) for c in range(NCH)]
    TT = sum(T_low) + sum(T_high)
    SLOTS = TT * P

    # Padded slots gather row 0 (dstloc=-1 masks them out of the selection
    # matrices).  The SWDGE reg contract requires num_idxs_reg == count of
    # non-negative indices per core, and the instruction stream is
    # SPMD-uniform, so -1 (skipped) indices can't be used for padding.
    idx16 = np.zeros((NC, SLOTS), dtype=np.int16)
    dstloc = np.full((NC, SLOTS), -1.0, dtype=np.float32)
    invd = np.zeros((NC, SLOTS), dtype=np.float32)

    # Layer order within the slot array: all chunks' LOW segments first
    # (pass A), then all chunks' HIGH segments (pass B).
    HIR = NPC - HALFR
    for i in range(NC):
        seg_data = {}
        for c in range(NCH):
            e0, e1 = chunk_lo[i, c], chunk_hi[i, c]
            seg_src = s_src[e0:e1]
            seg_dst = s_dst[e0:e1]
            s_i, s_r = seg_src // NPC, seg_src % NPC
            is_lo = s_r < HALFR
            tidx = np.where(is_lo, s_i * HALFR + s_r, s_i * HIR + (s_r - HALFR))
            base = i * NPC + c * P
            for half_sel in (0, 1):
                m = is_lo if half_sel == 0 else ~is_lo
                seg_data[(half_sel, c)] = (
                    tidx[m].astype(np.int16),
                    (seg_dst[m] - base).astype(np.float32),
                    invdeg_per_node[seg_dst[m]],
                )
        pos = 0
        for half_sel in (0, 1):
            T_pads = T_low if half_sel == 0 else T_high
            for c in range(NCH):
                ss, dd, vv = seg_data[(half_sel, c)]
                n = len(ss)
                t_pad = T_pads[c]
                assert n <= t_pad * P
                idx16[i, pos : pos + n] = ss
                dstloc[i, pos : pos + n] = dd
                invd[i, pos : pos + n] = vv
                pos += t_pad * P
        assert pos == SLOTS

    # SBUF layouts:
    #  idx16_sb [128, SLOTS//16]: per gather segment, slot j -> [j%16, j//16],
    #    replicated across the eight 16-partition groups.  Segments are
    #    multiples of 128 slots, so the per-segment wrap equals a global wrap.
    idx_w = idx16.reshape(NC, SLOTS // 16, 16).transpose(0, 2, 1)  # [NC,16,cols]
    idx16_sb = np.ascontiguousarray(np.tile(idx_w, (1, 8, 1)))  # [NC,128,cols]
    #  dstloc/invd [128, TT]: slot j -> [j%128, j//128]
    dstloc_sb = np.ascontiguousarray(dstloc.reshape(NC, TT, P).transpose(0, 2, 1))
    invd_sb = np.ascontiguousarray(invd.reshape(NC, TT, P).transpose(0, 2, 1))

    return T_low, T_high, idx16_sb, dstloc_sb, invd_sb


def table_permute(cfg: GSCfg, x: np.ndarray) -> np.ndarray:
    """Reorder node rows into the core-major-half gather-table layout."""
    g = np.arange(cfg.N)
    i, r = g // cfg.NPC, g % cfg.NPC
    hr, hir = cfg.HALFR, cfg.NPC - cfg.HALFR
    gp = np.where(r < hr, i * hr + r, cfg.HALF + i * hir + (r - hr))
    out = np.empty_like(x)
    out[gp] = x[g]
    return out


def build_program(cfg: GSCfg, T_low, T_high, has_bias, skip_collectives=False):
    """Build the SPMD Bass program (identical instruction stream per core)."""
    N, NPC, NCH, HALF, HALFR = cfg.N, cfg.NPC, cfg.NCH, cfg.HALF, cfg.HALFR
    D_IN, D_HID, D_OUT = cfg.D_IN, cfg.D_HID, cfg.D_OUT
    TT = sum(T_low) + sum(T_high)
    TMAX = max(max(T_low), max(T_high))
    HIR = NPC - HALFR
    # pass-B chunk index after which all of h_own[0:HALFR] has been written
    C_LOW_DONE = (HALFR + P - 1) // P - 1

    nc = bacc.Bacc(
        "TRN2",
        target_bir_lowering=False,
        debug=not axon_active(),
        num_devices=cfg.NCORES,
    )

    xtab = nc.dram_tensor("xtab", [N, D_IN], FP16, kind="ExternalInput")
    xown = nc.dram_tensor("xown", [NPC, D_IN], FP16, kind="ExternalInput")
    idx_d = nc.dram_tensor("idx16", [P, TT * 8], I16, kind="ExternalInput")
    dst_d = nc.dram_tensor("dstloc", [P, TT], F32, kind="ExternalInput")
    inv_d = nc.dram_tensor("invd", [P, TT], F32, kind="ExternalInput")
    w_d = {}
    for li, (din, dout) in enumerate(((D_IN, D_HID), (D_HID, D_HID), (D_HID, D_OUT))):
        w_d[f"wl{li}"] = nc.dram_tensor(f"wl{li}", [din, dout], FP16, kind="ExternalInput")
        w_d[f"wr{li}"] = nc.dram_tensor(f"wr{li}", [din, dout], FP16, kind="ExternalInput")
        if has_bias:
            w_d[f"b{li}"] = nc.dram_tensor(f"b{li}", [P, dout], F32, kind="ExternalInput")
    out_d = nc.dram_tensor("out", [NPC, D_OUT], F32, kind="ExternalOutput")

    from contextlib import ExitStack

    with tile.TileContext(nc) as tc, ExitStack() as stk:
        # ---- constants / static SBUF residents ----
        const = stk.enter_context(tc.tile_pool(name="const", bufs=1))
        iota_i = const.tile([P, P], mybir.dt.int32, name="iota_i")
        nc.gpsimd.iota(iota_i[:], pattern=[[1, P]], base=0, channel_multiplier=0)
        iota_h = const.tile([P, P], FP16, name="iota_h")
        nc.vector.tensor_copy(iota_h[:], iota_i[:])
        ident = const.tile([P, P], FP16, name="ident")
        make_identity(nc, ident[:])

        idx_t = const.tile([P, TT * 8], I16, name="idx_t")
        nc.sync.dma_start(idx_t[:], idx_d[:])
        dst_t = const.tile([P, TT], F32, name="dst_t")
        nc.sync.dma_start(dst_t[:], dst_d[:])
        inv_t = const.tile([P, TT], F32, name="inv_t")
        nc.sync.dma_start(inv_t[:], inv_d[:])

        w_t = {}
        for k, d in w_d.items():
            w_t[k] = const.tile(list(d.shape), d.dtype, name=f"{k}_t")
            nc.sync.dma_start(w_t[k][:], d[:])

        # per-chunk low-half partial aggregations (persistent across a layer)
        lop = stk.enter_context(tc.tile_pool(name="lop", bufs=1))
        loPart = [lop.tile([P, P], FP16, name=f"loPart{c}") for c in range(NCH)]

        # ---- inter-layer DRAM tables ----
        dram = stk.enter_context(tc.tile_pool(name="dram", bufs=1, space="DRAM"))
        h_own = [dram.tile([NPC, D_HID], FP16, name=f"h_own{li}") for li in range(2)]
        h_full = [
            (
                dram.tile([HALF, D_HID], FP16, name=f"h_full{li}_0", addr_space="Shared"),
                dram.tile([N - HALF, D_HID], FP16, name=f"h_full{li}_1", addr_space="Shared"),
            )
            for li in range(2)
        ]

        # ---- working pools ----
        msgp = stk.enter_context(tc.tile_pool(name="msg", bufs=3))
        sp = stk.enter_context(tc.tile_pool(name="sel", bufs=4))
        wk = stk.enter_context(tc.tile_pool(name="wk", bufs=3))
        ps_ag = stk.enter_context(tc.tile_pool(name="ps_ag", bufs=2, space="PSUM"))
        ps_tr = stk.enter_context(tc.tile_pool(name="ps_tr", bufs=2, space="PSUM"))
        ps_h = stk.enter_context(tc.tile_pool(name="ps_h", bufs=2, space="PSUM"))

        # first-use safety: gather skips padded slots, so zero the message
        # buffers once (NaN * 0 selection would poison the PSUM accumulate)
        for _ in range(3):
            m0 = msgp.tile([P, TMAX, D_HID], FP16, tag="msg")
            nc.vector.memset(m0[:], 0)

        def gather(out_ap, tab_ap, col0, n_idx):
            for off in range(0, n_idx, MAXI):
                n = min(MAXI, n_idx - off)
                t0, t1 = off // P, (off + n) // P
                nc.gpsimd.dma_gather(
                    out_ap[:, t0:t1, :],
                    tab_ap,
                    idx_t[:, col0 + off // 16 : col0 + (off + n) // 16],
                    num_idxs=n,
                    num_idxs_reg=n,
                    elem_size=D_HID,
                    queue_num=0,
                )

        def accumulate(msg_t, agg_ps, til, T):
            """PE-accumulate the transposed selection aggregation for T tiles."""
            for t in range(T):
                s_t = sp.tile([P, P], FP16, tag="S")
                nc.vector.tensor_scalar(
                    s_t[:],
                    iota_h[:],
                    dst_t[:, til + t : til + t + 1],
                    inv_t[:, til + t : til + t + 1],
                    mybir.AluOpType.is_equal,
                    mybir.AluOpType.mult,
                )
                nc.tensor.matmul(
                    agg_ps[:],
                    lhsT=msg_t[:, t, :],
                    rhs=s_t[:],
                    start=(t == 0),
                    stop=(t == T - 1),
                )

        ACT = mybir.ActivationFunctionType
        for layer in range(3):
            dout = D_HID if layer < 2 else D_OUT
            if layer == 0:
                tab_lo, tab_hi = xtab[0:HALF, :], xtab[HALF:N, :]
                own = xown[:]
            else:
                tab_lo, tab_hi = h_full[layer - 1][0][:], h_full[layer - 1][1][:]
                own = h_own[layer - 1][:]
            wl_t = w_t[f"wl{layer}"]
            wr_t = w_t[f"wr{layer}"]

            # ---- pass A: low-half partial aggregation per chunk ----
            col = 0
            til = 0
            for c in range(NCH):
                Tl = T_low[c]
                if Tl == 0:
                    nc.vector.memset(loPart[c][:], 0)
                    continue
                msg_t = msgp.tile([P, TMAX, D_HID], FP16, tag="msg")
                gather(msg_t[:, :Tl, :], tab_lo, col, Tl * P)
                agg_ps = ps_ag.tile([P, P], F32, tag="agg")
                accumulate(msg_t, agg_ps, til, Tl)
                nc.scalar.activation(loPart[c][:], agg_ps[:], ACT.Copy)
                col += Tl * 8
                til += Tl

            # ---- pass B: high-half + combine + project ----
            for c in range(NCH):
                Th = T_high[c]
                nrows = min(P, NPC - c * P)

                hi_sb = wk.tile([P, P], FP16, tag="hi_sb")
                if Th:
                    msg_t = msgp.tile([P, TMAX, D_HID], FP16, tag="msg")
                    gather(msg_t[:, :Th, :], tab_hi, col, Th * P)
                    agg_ps = ps_ag.tile([P, P], F32, tag="agg")
                    accumulate(msg_t, agg_ps, til, Th)
                    nc.scalar.activation(hi_sb[:], agg_ps[:], ACT.Copy)
                    col += Th * 8
                    til += Th
                else:
                    nc.vector.memset(hi_sb[:], 0)

                # own-rows transpose (for the root-weight matmul)
                own_sb = wk.tile([P, D_HID], FP16, tag="own")
                if nrows < P:
                    nc.vector.memset(own_sb[:], 0)
                nc.sync.dma_start(own_sb[:nrows], own[c * P : c * P + nrows, :])
                xT_ps = ps_tr.tile([P, P], FP16, tag="xT")
                nc.tensor.transpose(xT_ps[:], own_sb[:], ident[:])
                xT = wk.tile([P, P], FP16, tag="xT_sb")
                nc.scalar.activation(xT[:], xT_ps[:], ACT.Copy)

                # h = relu((lo + hi) @ wl + own @ wr (+ b))
                h_ps = ps_h.tile([P, dout], F32, tag="h")
                nc.tensor.matmul(h_ps[:], lhsT=loPart[c][:], rhs=wl_t[:], start=True, stop=False)
                nc.tensor.matmul(h_ps[:], lhsT=hi_sb[:], rhs=wl_t[:], start=False, stop=False)
                nc.tensor.matmul(h_ps[:], lhsT=xT[:], rhs=wr_t[:], start=False, stop=True)

                if layer < 2:
                    h_sb = wk.tile([P, dout], FP16, tag="h_sb")
                    if has_bias:
                        nc.vector.tensor_tensor(
                            h_sb[:], h_ps[:], w_t[f"b{layer}"][:], mybir.AluOpType.add
                        )
                        nc.scalar.activation(h_sb[:], h_sb[:], ACT.Relu)
                    else:
                        nc.scalar.activation(h_sb[:], h_ps[:], ACT.Relu)
                    nc.sync.dma_start(
                        h_own[layer][c * P : c * P + nrows, :], h_sb[:nrows]
                    )
                else:
                    o_sb = wk.tile([P, dout], F32, tag="o_sb")
                    if has_bias:
                        nc.vector.tensor_tensor(
                            o_sb[:], h_ps[:], w_t[f"b{layer}"][:], mybir.AluOpType.add
                        )
                    else:
                        nc.scalar.activation(o_sb[:], h_ps[:], ACT.Copy)
                    nc.sync.dma_start(out_d[c * P : c * P + nrows, :], o_sb[:nrows])

                # fire the low-half AllGather as soon as its producer rows are
                # done; the high-half one at end of layer.  (collectives must
                # stay on the gpsimd queue: NRT straight-line ordering)
                if layer < 2 and not skip_collectives:
                    if c == C_LOW_DONE:
                        nc.gpsimd.collective_compute(
                            "AllGather",
                            mybir.AluOpType.bypass,
                            replica_groups=[list(range(cfg.NCORES))],
                            ins=[h_own[layer][0:HALFR, :]],
                            outs=[h_full[layer][0].opt()],
                        )
                    elif c == NCH - 1:
                        nc.gpsimd.collective_compute(
                            "AllGather",
                            mybir.AluOpType.bypass,
                            replica_groups=[list(range(cfg.NCORES))],
                            ins=[h_own[layer][HALFR:NPC, :]],
                            outs=[h_full[layer][1].opt()],
                        )

    nc.compile()
    return nc


def make_in_maps(cfg: GSCfg, inputs: dict, pre, has_bias):
    T_low, T_high, idx16_sb, dstloc_sb, invd_sb = pre
    x = np.asarray(inputs["x"], dtype=np.float32)
    x_h = x.astype(NP_FP16)
    xtab = table_permute(cfg, x_h)
    in_maps = []
    for i in range(cfg.NCORES):
        m = {
            "xtab": xtab,
            "xown": np.ascontiguousarray(x_h[i * cfg.NPC : (i + 1) * cfg.NPC]),
            "idx16": idx16_sb[i],
            "dstloc": dstloc_sb[i],
            "invd": invd_sb[i],
        }
        for li in range(3):
            m[f"wl{li}"] = np.asarray(inputs[f"w_l{li}"], np.float32).astype(NP_FP16)
            m[f"wr{li}"] = np.asarray(inputs[f"w_r{li}"], np.float32).astype(NP_FP16)
            if has_bias:
                b = np.asarray(inputs[f"b{li}"], dtype=np.float32)
                m[f"b{li}"] = np.tile(b[None, :], (P, 1))
        in_maps.append(m)
    return in_maps


def run(cfg: GSCfg, inputs: dict, trace: bool = False, tmpdir: str | None = None):
    """Preprocess, build, and run on the 8 cores; returns (out, results)."""
    ei = np.asarray(inputs["edge_index"])
    src = ei[0].astype(np.int64)
    dst = ei[1].astype(np.int64)

    pre = preprocess(cfg, src, dst)

    biases = [np.asarray(inputs[f"b{i}"], dtype=np.float32) for i in range(3)]
    has_bias = any(np.any(b != 0) for b in biases)

    nc = build_program(cfg, pre[0], pre[1], has_bias)
    in_maps = make_in_maps(cfg, inputs, pre, has_bias)

    results = run_bass_kernel_spmd(
        nc,
        in_maps,
        core_ids=list(range(cfg.NCORES)),
        trace=trace,
        tmpdir=tmpdir,
    )
    outs = [np.asarray(r["out"], dtype=np.float32) for r in results.results]
    return np.concatenate(outs, axis=0), results


def kernel(**inputs) -> np.ndarray:
    cfg = GSCfg(n_nodes=50000, n_cores=8, d_in=128, d_hid=128, d_out=64, half=25000)
    out, _ = run(cfg, inputs, trace=False)
    return out


if __name__ == "__main__":
    sys.path.insert(0, os.path.dirname(os.path.abspath(__file__)))
    import reference

    inputs = {k: np.asarray(v) for k, v in reference.setup_inputs().items()}
    expected = np.asarray(reference.reference(**inputs))
    actual = kernel(**inputs)
    err = np.abs(actual - expected)
    rel = np.linalg.norm(actual - expected) / np.linalg.norm(expected)
    print("max abs err", err.max(), "rel", rel)
